# revision 11
# baseline (speedup 1.0000x reference)
"""Trainium2 Bass kernel for nn_Attention_6554120093744.

Tensor-parallel across 8 NeuronCores: core i owns Q heads 4i..4i+3 and KV
head i (n_rep=4 group intact), wqkv rows / wo columns sharded by head, KV
caches sharded on the head axis. Attention is fully head-local; the partial
wo outputs are summed on the host (unshard step).

Self-contained: hardcodes all shapes from the problem spec.
"""

import math
import numpy as np
import ml_dtypes

import concourse.bass as bass
import concourse.bacc as bacc
import concourse.mybir as mybir
import concourse.tile as tile
from concourse import masks
from concourse.bass_utils import run_bass_kernel_spmd

F32 = mybir.dt.float32
BF16 = mybir.dt.bfloat16
NPBF = ml_dtypes.bfloat16
AT = mybir.AluOpType
AF = mybir.ActivationFunctionType
AX = mybir.AxisListType

N_HEADS, N_KV, HD, DIM = 32, 8, 128, 4096
MAX_S, B, S = 4096, 8, 32
EPS, ROPE_BASE, NEG = 1e-5, 10000.0, -1e30
NCORE = 8
QH = N_HEADS // NCORE      # 4 q heads per core
TOK = B * S                # 256
QS = QH * HD               # 512 (q cols per core)
ESZ = QS + 2 * HD          # 768 (qkv cols per core)
CH = 512                   # score chunk width (positions)
AVC = 128                  # AV chunk width (positions)
SCALE = 1.0 / math.sqrt(HD)

_CACHE = {}


def _host_prep(x, wqkv, wo, q_norm_w, k_norm_w, cache_k, cache_v, input_pos, mask):
    """Derive graph structure + per-core input arrays from the actual values."""
    x = np.asarray(x, np.float32)
    wqkv = np.asarray(wqkv, np.float32)
    wo = np.asarray(wo, np.float32)
    q_norm_w = np.asarray(q_norm_w, np.float32)
    k_norm_w = np.asarray(k_norm_w, np.float32)
    input_pos = np.asarray(input_pos)
    mask = np.asarray(mask)

    # --- structure from mask ---
    col_any = mask.any(axis=(0, 1, 2))  # [MAX_S]
    nz = np.nonzero(col_any)[0]
    assert nz.size > 0, "fully-masked attention unsupported"
    pl = int(nz[-1]) + 1
    pl = max(pl, int(input_pos.max()) + 1) if mask[..., input_pos].any() else pl
    nch = (pl + CH - 1) // CH
    widths = [min(CH, pl - ci * CH) for ci in range(nch)]
    nav = (pl + AVC - 1) // AVC
    nav_full = pl // AVC
    tw = pl - AVC * nav_full  # tail width (0 if pl % 128 == 0)

    partials = []   # ordered (b, ci)
    madds = []
    for b in range(B):
        mb = mask[b, 0]  # [S, MAX_S]
        for ci in range(nch):
            c0, w = ci * CH, widths[ci]
            sub = mb[:, c0:c0 + w]
            if not sub.all():
                add = np.where(sub, 0.0, NEG).astype(np.float32)  # [S, w]
                add = np.tile(add, (QH, 1))                        # [128, w]
                pad = np.zeros((QH * S, CH), np.float32)
                pad[:, :w] = add
                partials.append((b, ci))
                madds.append(pad)
    n_mask = len(partials)
    maskadd = np.stack(madds) if n_mask else None

    # --- scatter runs from input_pos ---
    pos = input_pos.astype(np.int64)
    assert len(np.unique(pos)) == len(pos), "duplicate input_pos unsupported"
    runs = []  # (p0, t0, ln) global
    i = 0
    while i < S:
        j = i
        while j + 1 < S and pos[j + 1] == pos[j] + 1:
            j += 1
        p0, t0, ln = int(pos[i]), i, j - i + 1
        if p0 < pl:  # clip to attended range
            ln = min(ln, pl - p0)
            runs.append((p0, t0, ln))
        i = j + 1
    kruns = []  # (ci, off, t0, ln) split at CH boundaries
    for (p0, t0, ln) in runs:
        p = p0
        while p < p0 + ln:
            ci = p // CH
            end = min((ci + 1) * CH, p0 + ln)
            kruns.append((ci, p - ci * CH, t0 + (p - p0), end - p))
            p = end

    # softmax-without-max-subtraction bound (scores are Cauchy-Schwarz bounded)
    bound = math.sqrt(HD) * float(np.abs(q_norm_w).max()) * float(np.abs(k_norm_w).max())
    assert bound < 80.0, f"score bound {bound} too large for expless softmax"

    sig = (pl, tuple(partials), tuple(runs))

    # --- per-core arrays (host relayout + bf16 cast only) ---
    xT = np.ascontiguousarray(x.reshape(TOK, DIM).T).astype(NPBF)  # [DIM, TOK]

    # rope tables with folded norm weights: 4 tables per (q|k)
    inv = (ROPE_BASE ** (-np.arange(0, HD, 2, dtype=np.float64) / HD))  # [64]
    ang = pos[:, None].astype(np.float64) * inv[None, :]                # [S, 64]
    cos, sin = np.cos(ang).astype(np.float32), np.sin(ang).astype(np.float32)

    def rope_tabs(w, nrep):
        we, wo_ = w[0::2], w[1::2]
        tabs = np.stack([cos * we, sin * wo_, sin * we, cos * wo_])  # [4, S, 64]
        tabs = np.tile(tabs, (1, B, 1))           # [4, TOK, 64]
        return np.tile(tabs, (1, 1, nrep)).astype(np.float32)  # [4, TOK, nrep*64]

    ropeq = rope_tabs(q_norm_w, QH)   # [4, TOK, 256]
    ropek = rope_tabs(k_norm_w, 1)    # [4, TOK, 64]

    per_core = []
    for i in range(NCORE):
        qrows = wqkv[QS * i: QS * (i + 1)]                       # [512, DIM]
        krow = wqkv[N_HEADS * HD + HD * i: N_HEADS * HD + HD * (i + 1)]
        vrow = wqkv[(N_HEADS + N_KV) * HD + HD * i: (N_HEADS + N_KV) * HD + HD * (i + 1)]
        wq_sh = np.concatenate([qrows, krow, vrow], axis=0)       # [768, DIM]
        wqT = np.ascontiguousarray(wq_sh.T).astype(NPBF)          # [DIM, 768]
        woT = np.ascontiguousarray(wo[:, QS * i: QS * (i + 1)].T).astype(NPBF)  # [512, DIM]
        kT = np.ascontiguousarray(cache_k[:, i].transpose(0, 2, 1)).astype(NPBF)  # [B,128,MAX_S]
        v = cache_v[:, i]                                         # [B, MAX_S, 128]
        vg = np.ascontiguousarray(
            v.reshape(B, MAX_S // CH, CH // AVC, AVC, HD).transpose(0, 3, 1, 2, 4)
            .reshape(B, AVC, MAX_S)).astype(NPBF)                 # [B, 128, MAX_S]
        m = {"xT": xT, "wqkvT": wqT, "woT": woT, "kT": kT, "vg": vg,
             "ropeq": ropeq, "ropek": ropek}
        if n_mask:
            m["maskadd"] = maskadd
        per_core.append(m)

    meta = dict(pl=pl, nch=nch, widths=widths, nav=nav, nav_full=nav_full, tw=tw,
                partials=partials, runs=runs, kruns=kruns, n_mask=n_mask)
    return sig, meta, per_core


def _build(meta):
    pl, nch, widths = meta["pl"], meta["nch"], meta["widths"]
    nav, nav_full, tw = meta["nav"], meta["nav_full"], meta["tw"]
    partials, runs, kruns, n_mask = (meta["partials"], meta["runs"],
                                     meta["kruns"], meta["n_mask"])
    pidx = {bc: i for i, bc in enumerate(partials)}

    nc = bacc.Bacc(None, target_bir_lowering=False)
    xT_e = nc.declare_dram_parameter("xT", [DIM, TOK], BF16, isOutput=False)
    wq_e = nc.declare_dram_parameter("wqkvT", [DIM, ESZ], BF16, isOutput=False)
    wo_e = nc.declare_dram_parameter("woT", [QS, DIM], BF16, isOutput=False)
    kt_e = nc.declare_dram_parameter("kT", [B, HD, MAX_S], BF16, isOutput=False)
    vg_e = nc.declare_dram_parameter("vg", [B, AVC, MAX_S], BF16, isOutput=False)
    rq_e = nc.declare_dram_parameter("ropeq", [4, TOK, QS // 2], F32, isOutput=False)
    rk_e = nc.declare_dram_parameter("ropek", [4, TOK, HD // 2], F32, isOutput=False)
    mk_e = (nc.declare_dram_parameter("maskadd", [n_mask, QH * S, CH], F32,
                                      isOutput=False) if n_mask else None)
    out_e = nc.declare_dram_parameter("out", [TOK, DIM], F32, isOutput=True)

    MS = bass.MemorySpace
    with tile.TileContext(nc) as tc:
        with (
            tc.tile_pool(name="const", bufs=1) as cp,
            tc.tile_pool(name="persist", bufs=1) as pp,
            tc.tile_pool(name="wstream", bufs=3) as wsp,
            tc.tile_pool(name="kv", bufs=2) as kvp,
            tc.tile_pool(name="work", bufs=2) as wp,
            tc.tile_pool(name="psb", bufs=4, space=MS.PSUM) as psb,
            tc.tile_pool(name="pss", bufs=2, space=MS.PSUM) as pss,
        ):
            id_f = cp.tile([128, 128], F32, tag="idf")
            id_b = cp.tile([128, 128], BF16, tag="idb")
            masks.make_identity(nc, id_f[:])
            masks.make_identity(nc, id_b[:])

            # persistent loads
            xt = []
            for d in range(DIM // 128):
                t = pp.tile([128, TOK], BF16, tag=f"xt{d}")
                nc.sync.dma_start(t[:], xT_e[d * 128:(d + 1) * 128, :])
                xt.append(t)
            wot = []
            for h in range(QH):
                t = pp.tile([128, DIM], BF16, tag=f"wot{h}")
                nc.sync.dma_start(t[:], wo_e[h * 128:(h + 1) * 128, :])
                wot.append(t)
            rq = [[None] * 2 for _ in range(4)]
            rk = [[None] * 2 for _ in range(4)]
            for t4 in range(4):
                for th in range(2):
                    a = cp.tile([128, QS // 2], F32, tag=f"rq{t4}{th}")
                    nc.sync.dma_start(a[:], rq_e[t4, th * 128:(th + 1) * 128, :])
                    rq[t4][th] = a
                    b_ = cp.tile([128, HD // 2], F32, tag=f"rk{t4}{th}")
                    nc.sync.dma_start(b_[:], rk_e[t4, th * 128:(th + 1) * 128, :])
                    rk[t4][th] = b_
            mk = []
            for i in range(n_mask):
                (bb, ci) = partials[i]
                w = widths[ci]
                t = cp.tile([128, w], F32, tag=f"mk{i}")
                nc.sync.dma_start(t[:], mk_e[i, :, :w])
                mk.append(t)

            # ---- QKV projection: qkv[th] [128 tok, 768] f32 ----
            pq = [[None, None], [None, None]]
            for th in range(2):
                for half in range(2):
                    pq[th][half] = psb.tile([128, 384], F32, tag="bank",
                                            name=f"pq{th}{half}")
            for d in range(DIM // 128):
                wt = wsp.tile([128, ESZ], BF16, tag="wq")
                nc.sync.dma_start(wt[:], wq_e[d * 128:(d + 1) * 128, :])
                for th in range(2):
                    for half in range(2):
                        nc.tensor.matmul(
                            pq[th][half][:],
                            xt[d][:, th * 128:(th + 1) * 128],
                            wt[:, half * 384:(half + 1) * 384],
                            start=(d == 0), stop=(d == DIM // 128 - 1))
            qkv = []
            for th in range(2):
                t = pp.tile([128, ESZ], F32, tag=f"qkv{th}")
                nc.scalar.copy(t[:, 0:384], pq[th][0][:])
                nc.vector.tensor_copy(t[:, 384:768], pq[th][1][:])
                qkv.append(t)

            # ---- RMSNorm (w folded into rope tables) + RoPE per tok-half ----
            qro, kro, vnb = [], [], []
            for th in range(2):
                stats = wp.tile([128, 16], F32, tag="stats")
                sq = wp.tile([128, HD], F32, tag="sq")
                for h in range(QH + 1):  # 4 q heads + 1 k head
                    nc.scalar.activation(sq[:], qkv[th][:, h * HD:(h + 1) * HD],
                                         AF.Square, accum_out=stats[:, h:h + 1])
                # rsqrt(mean + eps) = reciprocal(sqrt(sumsq/HD + eps))
                nc.vector.tensor_scalar(stats[:, 8:13], stats[:, 0:5],
                                        1.0 / HD, EPS, op0=AT.mult, op1=AT.add)
                nc.scalar.activation(stats[:, 0:5], stats[:, 8:13], AF.Sqrt)
                nc.vector.reciprocal(stats[:, 8:13], stats[:, 0:5])

                qs = wp.tile([128, QS], F32, tag="qs")
                ks = wp.tile([128, HD], F32, tag="ks")
                for h in range(QH):
                    nc.vector.tensor_scalar_mul(qs[:, h * HD:(h + 1) * HD],
                                                qkv[th][:, h * HD:(h + 1) * HD],
                                                stats[:, 8 + h:9 + h])
                nc.vector.tensor_scalar_mul(ks[:], qkv[th][:, QS:QS + HD],
                                            stats[:, 12:13])

                def rope(src, tabs, width, dst_tag):
                    half = width // 2
                    x1 = src[:, 0:width:2]
                    x2 = src[:, 1:width:2]
                    t1 = wp.tile([128, half], F32, tag=dst_tag + "t1")
                    t2 = wp.tile([128, half], F32, tag=dst_tag + "t2")
                    dst = pp.tile([128, width], F32, tag=dst_tag + str(th))
                    nc.vector.tensor_tensor(t1[:], x1, tabs[0][th][:], op=AT.mult)
                    nc.vector.tensor_tensor(t2[:], x2, tabs[1][th][:], op=AT.mult)
                    nc.vector.tensor_tensor(dst[:, 0:width:2], t1[:], t2[:], op=AT.subtract)
                    nc.vector.tensor_tensor(t1[:], x1, tabs[2][th][:], op=AT.mult)
                    nc.vector.tensor_tensor(t2[:], x2, tabs[3][th][:], op=AT.mult)
                    nc.vector.tensor_tensor(dst[:, 1:width:2], t1[:], t2[:], op=AT.add)
                    return dst

                qro.append(rope(qs, rq, QS, "qro"))
                kro.append(rope(ks, rk, HD, "kro"))
                vb = pp.tile([128, HD], BF16, tag=f"vnb{th}")
                nc.vector.tensor_copy(vb[:], qkv[th][:, QS + HD:ESZ])
                vnb.append(vb)

            # full-block transposes (base partition 0): qTf[th][h] and kTf[th]
            # hold [128 dim, 128 tok(4b)] for all 4 batches of the tok-half
            qTf = [[None] * QH for _ in range(2)]
            kTf = [None, None]
            for th in range(2):
                for h in range(QH):
                    tp = pss.tile([128, 128], F32, tag="tq", name=f"tpq{th}{h}")
                    nc.tensor.matmul(tp[:], qro[th][:, h * HD:(h + 1) * HD],
                                     id_f[:, :], is_transpose=True)
                    t = pp.tile([128, 128], BF16, tag=f"qTf{th}{h}",
                                name=f"qTf{th}{h}")
                    nc.vector.tensor_copy(t[:], tp[:])
                    qTf[th][h] = t
                tp = pss.tile([128, 128], F32, tag="tq", name=f"tpk{th}")
                nc.tensor.matmul(tp[:], kro[th][:], id_f[:, :], is_transpose=True)
                t = pp.tile([128, 128], BF16, tag=f"kTf{th}", name=f"kTf{th}")
                nc.vector.tensor_copy(t[:], tp[:])
                kTf[th] = t

            # ---- per-(b) attention ----
            aoT = []
            for h in range(QH):
                t = pp.tile([128, TOK], BF16, tag=f"aoT{h}")
                aoT.append(t)

            for b in range(B):
                th, bl = b // 4, b % 4
                r0 = bl * S  # partition row offset within tok-half tiles

                # qT_b [128 dim, 4h*32 tok] bf16 from the full-block transposes
                qT = wp.tile([128, QH * S], BF16, tag="qT")
                for h in range(QH):
                    nc.vector.tensor_copy(qT[:, h * S:(h + 1) * S],
                                          qTf[th][h][:, bl * S:(bl + 1) * S])
                # v_new rows rebased to partition 0 (DMA shifts partitions)
                v0s = []
                for ri, (p0, t0, ln) in enumerate(runs):
                    v0 = wp.tile([S, HD], BF16, tag="v0", name=f"v0_{b}_{ri}")
                    nc.sync.dma_start(v0[0:ln, :], vnb[th][r0 + t0:r0 + t0 + ln, :])
                    v0s.append(v0)

                # K cache big tile + scatter of new keys
                kt = kvp.tile([128, pl], BF16, tag="ktb")
                nc.sync.dma_start(kt[:], kt_e[b, :, 0:pl])
                for (ci, off, t0, ln) in kruns:
                    nc.vector.tensor_copy(kt[:, ci * CH + off: ci * CH + off + ln],
                                          kTf[th][:, bl * S + t0: bl * S + t0 + ln])

                # scores + exp per chunk
                E = pp.tile([128, pl], F32, tag="E", bufs=2)
                stats2 = wp.tile([128, 16], F32, tag="st2")
                for ci in range(nch):
                    w = widths[ci]
                    sc = psb.tile([128, w], F32, tag="bank")
                    nc.tensor.matmul(sc[:], qT[:], kt[:, ci * CH: ci * CH + w])
                    if (b, ci) in pidx:
                        nc.vector.tensor_tensor(sc[:], sc[:], mk[pidx[(b, ci)]][:, :w],
                                                op=AT.add)
                    nc.scalar.activation(E[:, ci * CH: ci * CH + w], sc[:], AF.Exp,
                                         scale=SCALE, accum_out=stats2[:, ci:ci + 1])
                nc.vector.tensor_reduce(stats2[:, 15:16], stats2[:, 0:nch],
                                        axis=AX.X, op=AT.add)
                recip = wp.tile([128, 1], F32, tag="recip")
                nc.vector.reciprocal(recip[:], stats2[:, 15:16])

                # extract new-token attn cols, then zero them in E
                anTs = []
                for ri, (p0, t0, ln) in enumerate(runs):
                    an = wp.tile([128, S], BF16, tag="an")
                    nc.vector.tensor_scalar_mul(an[:, 0:ln], E[:, p0:p0 + ln], recip[:])
                    anp = pss.tile([S, 128], BF16, tag="tq")
                    nc.tensor.matmul(anp[0:ln, :], an[:, 0:ln], id_b[:, :],
                                     is_transpose=True)
                    anT = wp.tile([S, 128], BF16, tag="anT")
                    nc.vector.tensor_copy(anT[0:ln, :], anp[0:ln, :])
                    anTs.append((anT, ri, ln))
                    nc.vector.memset(E[:, p0:p0 + ln], 0.0)

                # normalize all attn weights -> bf16
                A = pp.tile([128, pl], BF16, tag="A", bufs=2)
                nc.vector.tensor_scalar_mul(A[:], E[:], recip[:])

                # V cache big tile
                vt = kvp.tile([128, nav_full * AVC], BF16, tag="vtb")
                nc.sync.dma_start(vt[:], vg_e[b, :, 0:nav_full * AVC])
                if tw:
                    vtl = kvp.tile([tw, AVC], BF16, tag="vtl")
                    nc.sync.dma_start(vtl[:], vg_e[b, 0:tw,
                                                   nav_full * AVC:(nav_full + 1) * AVC])

                # attn @ V -> av [128 dim, 4h*32] accumulate
                av = pss.tile([128, QH * S], F32, tag="av")
                n_mm = nav + len(anTs)
                mi = 0
                for ai in range(nav):
                    aw = AVC if ai < nav_full else tw
                    atp = pss.tile([AVC, 128], BF16, tag="tq")
                    nc.tensor.matmul(atp[0:aw, :], A[:, ai * AVC: ai * AVC + aw],
                                     id_b[:, :], is_transpose=True)
                    atT = wp.tile([AVC, 128], BF16, tag="atT", bufs=3)
                    nc.vector.tensor_copy(atT[0:aw, :], atp[0:aw, :])
                    lhs = (vt[:, ai * AVC:(ai + 1) * AVC] if ai < nav_full
                           else vtl[0:aw, :])
                    nc.tensor.matmul(av[:], lhs, atT[0:aw, :],
                                     start=(mi == 0), stop=(mi == n_mm - 1))
                    mi += 1
                for (anT, ri, ln) in anTs:
                    nc.tensor.matmul(av[:], v0s[ri][0:ln, :], anT[0:ln, :],
                                     start=(mi == 0), stop=(mi == n_mm - 1))
                    mi += 1

                for h in range(QH):
                    nc.vector.tensor_copy(aoT[h][:, b * S:(b + 1) * S],
                                          av[:, h * S:(h + 1) * S])

            # ---- wo projection: y[tok, do] partial ----
            for th in range(2):
                for dc in range(DIM // 512):
                    yp = psb.tile([128, 512], F32, tag="bank")
                    for h in range(QH):
                        nc.tensor.matmul(yp[:], aoT[h][:, th * 128:(th + 1) * 128],
                                         wot[h][:, dc * 512:(dc + 1) * 512],
                                         start=(h == 0), stop=(h == QH - 1))
                    ys = wp.tile([128, 512], F32, tag="ys", bufs=3)
                    if dc % 2 == 0:
                        nc.vector.tensor_copy(ys[:], yp[:])
                    else:
                        nc.scalar.copy(ys[:], yp[:])
                    nc.sync.dma_start(out_e[th * 128:(th + 1) * 128,
                                            dc * 512:(dc + 1) * 512], ys[:])

    nc.compile()
    return nc


def _get(inputs):
    sig, meta, per_core = _host_prep(**inputs)
    if sig not in _CACHE:
        _CACHE[sig] = _build(meta)
    return _CACHE[sig], per_core


def kernel(**inputs) -> np.ndarray:
    nc, per_core = _get(inputs)
    res = run_bass_kernel_spmd(nc, per_core, core_ids=list(range(NCORE)))
    y = np.zeros((TOK, DIM), np.float64)
    for i in range(NCORE):
        y += res.results[i]["out"].astype(np.float64)
    return y.astype(np.float32).reshape(B, S, DIM)


def _make_runner(nc, in_maps):
    """Build the shard_map-jitted executable once (mirrors run_bass_via_pjrt)."""
    import jax
    from jax.sharding import Mesh, PartitionSpec
    from jax.experimental.shard_map import shard_map
    from concourse import bass2jax

    bass2jax.install_neuronx_cc_hook()
    n_cores = len(in_maps)
    partition_name = nc.partition_id_tensor.name if nc.partition_id_tensor else None
    in_names, out_names, out_avals, zero_outs = [], [], [], []
    for alloc in nc.m.functions[0].allocations:
        if not isinstance(alloc, mybir.MemoryLocationSet):
            continue
        name = alloc.memorylocations[0].name
        if alloc.kind == "ExternalInput":
            if name != partition_name:
                in_names.append(name)
        elif alloc.kind == "ExternalOutput":
            shape = tuple(alloc.tensor_shape)
            dtype = mybir.dt.np(alloc.dtype)
            out_names.append(name)
            out_avals.append(jax.core.ShapedArray(shape, dtype))
            zero_outs.append(np.zeros(shape, dtype))
    n_params = len(in_names)
    all_names = in_names + out_names
    if partition_name is not None:
        all_names = all_names + [partition_name]

    def _body(*args):
        operands = list(args)
        if partition_name is not None:
            operands.append(bass2jax.partition_id_tensor())
        outs = bass2jax._bass_exec_p.bind(
            *operands,
            out_avals=tuple(out_avals),
            in_names=tuple(all_names),
            out_names=tuple(out_names),
            lowering_input_output_aliases=(),
            sim_require_finite=True,
            sim_require_nnan=True,
            nc=nc,
        )
        return tuple(outs)

    devices = jax.devices()[:n_cores]
    mesh = Mesh(np.asarray(devices), ("core",))
    in_specs = (PartitionSpec("core"),) * (n_params + len(out_names))
    out_specs = (PartitionSpec("core"),) * len(out_names)
    fn = jax.jit(shard_map(_body, mesh=mesh, in_specs=in_specs,
                           out_specs=out_specs, check_rep=False), keep_unused=True)
    concat_in = [np.concatenate([np.asarray(in_maps[c][n]) for c in range(n_cores)],
                                axis=0) for n in in_names]
    concat_zero = [np.zeros((n_cores * z.shape[0], *z.shape[1:]), z.dtype)
                   for z in zero_outs]
    args = [jax.device_put(a) for a in concat_in + concat_zero]
    return fn, args


def time_kernel(nc, in_maps, n1=5, n2=25):
    """Per-execution time via async-pipelined repeats, two-point slope."""
    import time, jax
    fn, args = _make_runner(nc, in_maps)

    def run_n(n):
        outs = None
        t0 = time.perf_counter()
        for _ in range(n):
            outs = fn(*args)
        jax.block_until_ready(outs)
        return time.perf_counter() - t0

    run_n(3)  # warmup (compile + cache)
    t1 = min(run_n(n1) for _ in range(3))
    t2 = min(run_n(n2) for _ in range(3))
    per = (t2 - t1) / (n2 - n1)
    return per * 1e9
